# revision 15
# baseline (speedup 1.0000x reference)
"""Trainium2 Bass kernel for nn_LinearLayer_45243185496808.

Computes out[b,o] = sum_i tanh(x[b,i]*t) * (sum_p coef[o,i,p]) with
B=131072, I=O=128, P_NUM=16, data-parallel over batch on 8 NeuronCores.

The kernel is elementwise-walled: ScalarE tanh (1x @1.2GHz) plus the
PSUM->SBUF fp32 eviction (1x on DVE @0.96GHz / ScalarE) are ~16-17us of
combined engine time that nothing else can absorb (GpSimd has no PSUM
port, DMA can't read PSUM, TRN2 matmul output must be fp32). Schedule
highlights, all trace-derived:

  - x ships as fp8e4 [i=128, b] (pure transpose, 1B/elem): 2 MiB/core,
    ACTIVATE reads fp8 at the same 1x rate. End-to-end rel err ~1.5e-2
    vs the 2e-2 gate (dominated by fp8 quantization of x).
  - transposed output: wT [i,o] f16 is the PE stationary; tanh values
    [i,b] f16 stream as the moving operand in N=512 blocks. PSUM holds
    out.T; host transposes the stored [128, B] f16 back.
  - HAM clock: the PE runs N=512 matmuls at 215ns only in the k=8/8
    activity state and ~630ns at k=4 (measured). Warmup matmuls on a
    zero scratch run from kernel start until the coef DMA lands so the
    PE hits k=8 before the w-reduction, and trailing dummy matmuls keep
    it there through the store drain + the framework's fixed per-engine
    semaphore-reset storm (whose Tensor-side resets run 2x faster at
    k=8 — the storm is ~half the measured tail).
  - loads alternate sync/gpsimd rings so chunk sems arrive at ~0.6us
    spacing despite the ~2.5-3us per-DMA completion latency; the coef
    (with an identity block for the w = sum_p coef reduction via 16
    accumulating identity matmuls) is the first issue on each ring.
  - PSUM = one [1536] x2 ring (6 banks) for the main chunks + one
    [512] x2 ring (2 banks) shared by warmup/w/dummies and the tail
    chunks, so the tail's matmuls never queue behind DVE's eviction
    backlog on the big ring.
  - eviction split: DVE takes ~14.3k cols; ScalarE takes the final 2048
    (c5 tail + c6 + c7) emitted after the last tanh, so both engines
    drain together; stores ride gpsimd/SWDGE except the last chunks on
    the by-then-idle sync/HWDGE ring.

HBM per core: 2 MiB x(fp8) + 0.53 MiB coef+identity + 4 MiB out(f16).
"""

import os
import sys
import types

import ml_dtypes
import numpy as np

import concourse.bass as bass
import concourse.mybir as mybir
import concourse.tile as tile
from concourse import bacc
from concourse.bass_utils import run_bass_kernel_spmd


def _ensure_ntff_hook():
    """Register the axon NTFF profile hook if the image lacks antenv.axon_hooks.

    Only needed for BASS_TRACE=1 profiling runs; harmless otherwise."""
    if "antenv.axon_hooks" in sys.modules:
        return
    try:
        from antenv.axon_hooks import get_axon_ntff_profile_hook  # noqa: F401

        return  # real module importable
    except ImportError:
        pass
    hook = None
    try:
        from trn_agent_boot.trn_boot import _ntff_profile_via_ctypes

        so_path = "/opt/axon/libaxon_pjrt.so"
        if os.path.exists(so_path):
            hook = _ntff_profile_via_ctypes(so_path)
    except Exception:
        hook = None
    mod = types.ModuleType("antenv.axon_hooks")
    mod.get_axon_ntff_profile_hook = lambda: hook
    mod.set_axon_ntff_profile_hook = lambda h: None
    sys.modules["antenv.axon_hooks"] = mod


N_CORES = 8
B_FULL = 131072
I_DIM = 128
O_DIM = 128
P_NUM = 16
P = 128                     # SBUF partitions
B_CORE = B_FULL // N_CORES  # 16384

# Load chunks of xt == tanh tiles. Fine taper up front (load-completion
# sems lag ~5us behind issue once several DMAs queue on a ring, so the
# early chunks must be small to arrive in tanh order), big middle
# (amortize the 352-cycle ACTIVATE overhead), taper down (short drain).
WIDTHS = [1024, 1536, 2048, 3072, 4096, 2048, 1536, 1024]
assert sum(WIDTHS) == B_CORE
CHUNKS = []
_b = 0
for _w in WIDTHS:
    CHUNKS.append((_b, _w))
    _b += _w
N_TAIL = 1                  # last chunk: tail PSUM ring + ScalarE eviction
N_DEFER = 2                 # ...plus the last 512 of this many prior chunks

NMM = 512                   # moving cols per matmul = one PSUM bank of f32
MAIN_T = 1536               # main PSUM tile cols (3 banks), x2 bufs
TAIL_T = 512                # tail PSUM tile cols (1 bank), x2 bufs
N_WARM = 24                 # PE warmup matmuls (fill until coef lands)
N_DUMMY = 10                # trailing PE matmuls (hold k=8 through drain)
# PE filler matmuls emitted after each chunk's real matmuls: the HAM
# activity manager drops the NC to the k=4 clock state after ~1.7us of
# PE idle, which halves matmul rate AND slows DVE/ScalarE ~10%; filler
# keeps the PE stream dense through the tanh-paced gaps.
FILLERS = {0: 3, 1: 4, 2: 5, 3: 7, 4: 9, 5: 4, 6: 3}

# coefT layout: [identity(128) | p-major coef blocks (16 x 128)]
CW = O_DIM * P_NUM          # 2048
COEF_COLS = 128 + CW        # 2176
HALF_A = 128 + CW // 2      # identity + blocks 0-7 -> sync ring

LAST_RESULT = None  # BassKernelResults of the most recent run (for test.py)


def build_bass(tanh_scale: float) -> bass.Bass:
    nc = bacc.Bacc("TRN2", target_bir_lowering=False)
    xt = nc.dram_tensor("xt", [P, B_CORE], mybir.dt.float8e4, kind="ExternalInput")
    coefT = nc.dram_tensor(
        "coefT", [I_DIM, COEF_COLS], mybir.dt.float16, kind="ExternalInput"
    )
    outT = nc.dram_tensor("outT", [P, B_CORE], mybir.dt.float16, kind="ExternalOutput")

    with tile.TileContext(nc) as tc:
        with (
            tc.tile_pool(name="consts", bufs=1) as consts,
            tc.tile_pool(name="xin", bufs=4) as xin_pool,
            tc.tile_pool(name="vals", bufs=4) as vals_pool,
            tc.tile_pool(name="outp", bufs=4) as out_pool,
            tc.tile_pool(name="pout", bufs=2, space="PSUM") as pout_pool,
        ):
            # Zero scratch for PE warmup, memset on the otherwise-idle DVE
            # so the PE can start ramping immediately.
            warm = consts.tile([P, P], mybir.dt.float16)
            nc.vector.memset(warm[:], 0.0)

            def tail_tile():
                return pout_pool.tile(
                    [P, TAIL_T], mybir.dt.float32, tag="o_tail", name="o_tail"
                )

            for wi in range(N_WARM):
                t = tail_tile()
                nc.tensor.matmul(t[:, :P], warm[:], warm[:], start=True, stop=True)

            coef_sb = consts.tile([P, COEF_COLS], mybir.dt.float16)
            x_tiles = [None] * len(CHUNKS)

            def load_chunk(ci, eng):
                base, wcols = CHUNKS[ci]
                # Shared 4-deep ring: chunk k's dma_start blocks until the
                # tile from chunk k-4 is consumed by its tanh. Queuing all
                # loads at once measured ~4us completion-sem bunching (the
                # 16 SDMA engines round-robin all queued descriptors, so
                # every queued DMA finishes near the end of the whole load
                # phase); the ring caps in-flight loads at ~3.
                x_sb = xin_pool.tile([P, wcols], mybir.dt.float8e4, tag="x")
                eng.dma_start(out=x_sb[:], in_=xt[:, base : base + wcols])
                x_tiles[ci] = x_sb

            # All loads ride the sync HWDGE ring (SWDGE/gpsimd loads
            # measured ~6us issue->sem vs ~2.5 on sync). coef half A
            # (identity + blocks 0-7) first — wT gates the whole eviction
            # stream — then chunk 0, coef half B, and the rest.
            nc.sync.dma_start(out=coef_sb[:, :HALF_A], in_=coefT[:, :HALF_A])
            load_chunk(0, nc.sync)
            nc.sync.dma_start(out=coef_sb[:, HALF_A:], in_=coefT[:, HALF_A:])
            for ci in range(1, len(CHUNKS)):
                load_chunk(ci, nc.sync)

            # wT[i,o] = sum_p coef via 16 identity matmuls accumulating in
            # PSUM (I.T @ block_p = block_p), then one DVE cast to f16.
            identity_h = coef_sb[:, :P]
            w_big = tail_tile()
            w_ps = w_big[:, :O_DIM]
            for k in range(P_NUM):
                nc.tensor.matmul(
                    w_ps,
                    identity_h,
                    coef_sb[:, P + k * O_DIM : P + (k + 1) * O_DIM],
                    start=(k == 0),
                    stop=(k == P_NUM - 1),
                )
            # wT cast on ScalarE: it lands in ScalarE's load-stall window
            # between tanh0 and tanh1 and keeps DVE's queue pure-eviction.
            wT = consts.tile([P, O_DIM], mybir.dt.float16)
            nc.scalar.copy(wT[:], w_ps)

            # --- main pipeline ---
            deferred = []  # ScalarE-drain evictions: (out_sb slice, psum)
            stores = []    # deferred stores: (ci, base, wcols, out_sb)

            for ci, (base, wcols) in enumerate(CHUNKS):
                tail = ci >= len(CHUNKS) - N_TAIL
                v_sb = vals_pool.tile([P, wcols], mybir.dt.float16, tag="v_sb")
                nc.scalar.activation(
                    v_sb[:],
                    x_tiles[ci][:],
                    mybir.ActivationFunctionType.Tanh,
                    scale=tanh_scale,
                )
                out_sb = out_pool.tile([P, wcols], mybir.dt.float16, tag="out_sb")
                # Tile widths: tail chunks all [512] on the tail ring; the
                # last N_DEFER main chunks end in a [512] slice deferred to
                # ScalarE.
                split_last = (
                    len(CHUNKS) - N_TAIL - N_DEFER <= ci < len(CHUNKS) - N_TAIL
                )
                if tail:
                    widths = [TAIL_T] * (wcols // TAIL_T)
                else:
                    widths, rem = [], wcols - (TAIL_T if split_last else 0)
                    while rem > 0:
                        widths.append(min(MAIN_T, rem))
                        rem -= widths[-1]
                    if split_last:
                        widths.append(TAIL_T)
                g0 = 0
                for ti, gw in enumerate(widths):
                    if tail:
                        o_ps = tail_tile()
                    else:
                        o_ps = pout_pool.tile([P, gw], mybir.dt.float32, tag="o_ps")
                        o_ps = o_ps[:]
                    for j0 in range(0, gw, NMM):
                        jw = min(NMM, gw - j0)
                        nc.tensor.matmul(
                            o_ps[:, j0 : j0 + jw],
                            wT[:],
                            v_sb[:, g0 + j0 : g0 + j0 + jw],
                            start=True,
                            stop=True,
                        )
                    # ScalarE (after its tanh stream ends) takes the tail
                    # chunk plus the last 512 of the N_DEFER chunks before
                    # it; DVE takes the rest.
                    if tail or (split_last and ti == len(widths) - 1):
                        deferred.append((out_sb[:, g0 : g0 + gw], o_ps))
                    else:
                        nc.vector.tensor_copy(out_sb[:, g0 : g0 + gw], o_ps)
                    g0 += gw
                # Filler matmuls keep the PE stream dense (HAM k=8).
                for _ in range(FILLERS.get(ci, 0)):
                    t = tail_tile()
                    nc.tensor.matmul(
                        t[:, :P], warm[:], warm[:], start=True, stop=True
                    )
                if ci < len(CHUNKS) - N_TAIL - N_DEFER:
                    nc.gpsimd.dma_start(
                        out=outT[:, base : base + wcols], in_=out_sb[:]
                    )
                else:
                    # Chunks with a deferred eviction store only after the
                    # ScalarE drain completes their out tile.
                    stores.append((base, wcols, out_sb))

            # ScalarE drain (tanh done): evict tail slices, store on the
            # idle sync ring.
            for dst, o_ps in deferred:
                nc.scalar.copy(dst, o_ps)
            for base, wcols, out_sb in stores:
                nc.sync.dma_start(out=outT[:, base : base + wcols], in_=out_sb[:])

            # Hold the PE's k=8 activity state through the store drain and
            # into the framework's semaphore-reset storm (its Tensor-side
            # resets run ~2x faster at k=8).
            for wi in range(N_DUMMY):
                t = tail_tile()
                nc.tensor.matmul(t[:, :P], warm[:], warm[:], start=True, stop=True)
    nc.finalize()
    return nc


def kernel(x, coef, tanh_range):
    global LAST_RESULT
    x = np.asarray(x, dtype=np.float32)
    coef = np.asarray(coef, dtype=np.float32)
    t = float(np.asarray(tanh_range))
    assert x.shape == (B_FULL, I_DIM), x.shape
    assert coef.shape == (O_DIM, I_DIM, P_NUM), coef.shape

    # [identity | p-major coef blocks]: block p is the [i, o] slice.
    coefT = np.empty((I_DIM, COEF_COLS), dtype=np.float16)
    coefT[:, :P] = np.eye(P, dtype=np.float16)
    coefT[:, P:] = (
        coef.transpose(1, 2, 0).astype(np.float16).reshape(I_DIM, CW)
    )
    nc = build_bass(t)
    xt_full = np.ascontiguousarray(x.T).astype(ml_dtypes.float8_e4m3)
    in_maps = [
        {"xt": np.ascontiguousarray(xt_full[:, k * B_CORE : (k + 1) * B_CORE]),
         "coefT": coefT}
        for k in range(N_CORES)
    ]
    if os.environ.get("BASS_TRACE"):
        _ensure_ntff_hook()
    res = run_bass_kernel_spmd(nc, in_maps, core_ids=list(range(N_CORES)))
    LAST_RESULT = res
    return np.concatenate(
        [r["outT"].astype(np.float32).T for r in res.results], axis=0
    )


# revision 19
# speedup vs baseline: 1.1234x; 1.1234x over previous
"""Trainium2 Bass kernel for nn_LinearLayer_45243185496808.

Computes out[b,o] = sum_i tanh(x[b,i]*t) * (sum_p coef[o,i,p]) with
B=131072, I=O=128, P_NUM=16, data-parallel over batch on 8 NeuronCores.

The kernel is elementwise-walled: ScalarE tanh (1x @1.2GHz) plus the
PSUM->SBUF fp32 eviction (1x on DVE @0.96GHz / ScalarE) are ~16-17us of
combined engine time that nothing else can absorb (GpSimd has no PSUM
port, DMA can't read PSUM, TRN2 matmul output must be fp32). Schedule
highlights, all trace-derived:

  - x ships as fp8e4 [i=128, b] (pure transpose, 1B/elem): 2 MiB/core,
    ACTIVATE reads fp8 at the same 1x rate. End-to-end rel err ~1.5e-2
    vs the 2e-2 gate (dominated by fp8 quantization of x).
  - transposed output: wT [i,o] f16 is the PE stationary; tanh values
    [i,b] f16 stream as the moving operand in N=512 blocks. PSUM holds
    out.T; host transposes the stored [128, B] f16 back.
  - HAM clock: the PE runs N=512 matmuls at 215ns only in the k=8/8
    activity state and ~630ns at k=4 (measured). Warmup matmuls on a
    zero scratch run from kernel start until the coef DMA lands so the
    PE hits k=8 before the w-reduction, and trailing dummy matmuls keep
    it there through the store drain + the framework's fixed per-engine
    semaphore-reset storm (whose Tensor-side resets run 2x faster at
    k=8 — the storm is ~half the measured tail).
  - loads alternate sync/gpsimd rings so chunk sems arrive at ~0.6us
    spacing despite the ~2.5-3us per-DMA completion latency; the coef
    (with an identity block for the w = sum_p coef reduction via 16
    accumulating identity matmuls) is the first issue on each ring.
  - PSUM = one [1536] x2 ring (6 banks) for the main chunks + one
    [512] x2 ring (2 banks) shared by warmup/w/dummies and the tail
    chunks, so the tail's matmuls never queue behind DVE's eviction
    backlog on the big ring.
  - eviction split: DVE takes ~14.3k cols; ScalarE takes the final 2048
    (c5 tail + c6 + c7) emitted after the last tanh, so both engines
    drain together; stores ride gpsimd/SWDGE except the last chunks on
    the by-then-idle sync/HWDGE ring.

HBM per core: 2 MiB x(fp8) + 0.53 MiB coef+identity + 4 MiB out(f16).
"""

import os
import sys
import types

import ml_dtypes
import numpy as np

import concourse.bass as bass
import concourse.mybir as mybir
import concourse.tile as tile
from concourse import bacc
from concourse.bass_utils import run_bass_kernel_spmd


def _ensure_ntff_hook():
    """Register the axon NTFF profile hook if the image lacks antenv.axon_hooks.

    Only needed for BASS_TRACE=1 profiling runs; harmless otherwise."""
    if "antenv.axon_hooks" in sys.modules:
        return
    try:
        from antenv.axon_hooks import get_axon_ntff_profile_hook  # noqa: F401

        return  # real module importable
    except ImportError:
        pass
    hook = None
    try:
        from trn_agent_boot.trn_boot import _ntff_profile_via_ctypes

        so_path = "/opt/axon/libaxon_pjrt.so"
        if os.path.exists(so_path):
            hook = _ntff_profile_via_ctypes(so_path)
    except Exception:
        hook = None
    mod = types.ModuleType("antenv.axon_hooks")
    mod.get_axon_ntff_profile_hook = lambda: hook
    mod.set_axon_ntff_profile_hook = lambda h: None
    sys.modules["antenv.axon_hooks"] = mod


N_CORES = 8
B_FULL = 131072
I_DIM = 128
O_DIM = 128
P_NUM = 16
P = 128                     # SBUF partitions
B_CORE = B_FULL // N_CORES  # 16384

# Load chunks of xt == tanh tiles. Fine taper up front (load-completion
# sems lag ~5us behind issue once several DMAs queue on a ring, so the
# early chunks must be small to arrive in tanh order), big middle
# (amortize the 352-cycle ACTIVATE overhead), taper down (short drain).
WIDTHS = [512, 1024, 1536, 2560, 4096, 3584, 1536, 1024, 512]
assert sum(WIDTHS) == B_CORE
CHUNKS = []
_b = 0
for _w in WIDTHS:
    CHUNKS.append((_b, _w))
    _b += _w
N_TAIL = 2                  # last chunks: tail PSUM ring + ScalarE eviction
N_DEFER = 1                 # ...plus the last 512 of this many prior chunks

NMM = 512                   # moving cols per matmul = one PSUM bank of f32
MAIN_T = 1536               # main PSUM tile cols (3 banks), x2 bufs
TAIL_T = 512                # tail PSUM tile cols (1 bank), x2 bufs
N_WARM = 24                 # PE warmup matmuls (fill until coef lands)
N_DUMMY = 10                # trailing PE matmuls (hold k=8 through drain)
# PE filler matmuls emitted after each chunk's real matmuls: the HAM
# activity manager drops the NC to the k=4 clock state after ~1.7us of
# PE idle, which halves matmul rate AND slows DVE/ScalarE ~10%; filler
# keeps the PE stream dense through the tanh-paced gaps.
FILLERS = {0: 2, 1: 3, 2: 4, 3: 5, 4: 7, 5: 7, 6: 3, 7: 2}

# coefT layout: [identity(128) | p-major coef blocks (16 x 128)]
CW = O_DIM * P_NUM          # 2048
COEF_COLS = 128 + CW        # 2176
HALF_A = 128 + CW // 2      # identity + blocks 0-7 -> sync ring

LAST_RESULT = None  # BassKernelResults of the most recent run (for test.py)


def build_bass(tanh_scale: float) -> bass.Bass:
    nc = bacc.Bacc("TRN2", target_bir_lowering=False)
    xt = nc.dram_tensor("xt", [P, B_CORE], mybir.dt.float8e4, kind="ExternalInput")
    coefT = nc.dram_tensor(
        "coefT", [I_DIM, COEF_COLS], mybir.dt.float16, kind="ExternalInput"
    )
    outT = nc.dram_tensor("outT", [P, B_CORE], mybir.dt.float16, kind="ExternalOutput")

    with tile.TileContext(nc) as tc:
        with (
            tc.tile_pool(name="consts", bufs=1) as consts,
            tc.tile_pool(name="xin", bufs=1) as xin_pool,
            tc.tile_pool(name="vals", bufs=4) as vals_pool,
            tc.tile_pool(name="outp", bufs=4) as out_pool,
            tc.tile_pool(name="pout", bufs=2, space="PSUM") as pout_pool,
        ):
            # Zero scratch for PE warmup, memset on the otherwise-idle DVE
            # so the PE can start ramping immediately.
            warm = consts.tile([P, P], mybir.dt.float16)
            nc.vector.memset(warm[:], 0.0)

            def tail_tile():
                return pout_pool.tile(
                    [P, TAIL_T], mybir.dt.float32, tag="o_tail", name="o_tail"
                )

            for wi in range(N_WARM):
                t = tail_tile()
                nc.tensor.matmul(t[:, :P], warm[:], warm[:], start=True, stop=True)

            coef_sb = consts.tile([P, COEF_COLS], mybir.dt.float16)
            x_tiles = [None] * len(CHUNKS)

            def load_chunk(ci, eng):
                base, wcols = CHUNKS[ci]
                x_sb = xin_pool.tile([P, wcols], mybir.dt.float8e4, tag=f"x{ci}")
                eng.dma_start(out=x_sb[:], in_=xt[:, base : base + wcols])
                x_tiles[ci] = x_sb

            # Queued DMAs on one ring complete bunched near the end of the
            # whole queue (the 16 SDMA engines round-robin all queued
            # descriptors), so the two chunks the tanh stream needs first
            # after c0 ride the OTHER HWDGE ring (nc.scalar / ACT): its
            # issue slots precede ScalarE's ACT_TABLE_LOAD, the queue is
            # otherwise empty, and their sems land ~11-12us. Everything
            # else (coef halves first — wT gates the eviction stream —
            # then c0 and the big chunks) queues on sync.
            load_chunk(1, nc.scalar)
            load_chunk(2, nc.scalar)
            nc.sync.dma_start(out=coef_sb[:, :HALF_A], in_=coefT[:, :HALF_A])
            nc.sync.dma_start(out=coef_sb[:, HALF_A:], in_=coefT[:, HALF_A:])
            load_chunk(0, nc.sync)
            for ci in range(3, len(CHUNKS)):
                load_chunk(ci, nc.sync)

            # wT[i,o] = sum_p coef via 16 identity matmuls accumulating in
            # PSUM (I.T @ block_p = block_p), then one DVE cast to f16.
            identity_h = coef_sb[:, :P]
            w_big = tail_tile()
            w_ps = w_big[:, :O_DIM]
            for k in range(P_NUM):
                nc.tensor.matmul(
                    w_ps,
                    identity_h,
                    coef_sb[:, P + k * O_DIM : P + (k + 1) * O_DIM],
                    start=(k == 0),
                    stop=(k == P_NUM - 1),
                )
            wT = consts.tile([P, O_DIM], mybir.dt.float16)
            nc.vector.tensor_copy(wT[:], w_ps)

            # --- main pipeline ---
            deferred = []  # ScalarE-drain evictions: (out_sb slice, psum)
            stores = []    # deferred stores: (ci, base, wcols, out_sb)

            for ci, (base, wcols) in enumerate(CHUNKS):
                tail = ci >= len(CHUNKS) - N_TAIL
                v_sb = vals_pool.tile([P, wcols], mybir.dt.float16, tag="v_sb")
                nc.scalar.activation(
                    v_sb[:],
                    x_tiles[ci][:],
                    mybir.ActivationFunctionType.Tanh,
                    scale=tanh_scale,
                )
                out_sb = out_pool.tile([P, wcols], mybir.dt.float16, tag="out_sb")
                # Tile widths: tail chunks all [512] on the tail ring; the
                # last N_DEFER main chunks end in a [512] slice deferred to
                # ScalarE.
                split_last = (
                    len(CHUNKS) - N_TAIL - N_DEFER <= ci < len(CHUNKS) - N_TAIL
                )
                if tail:
                    widths = [TAIL_T] * (wcols // TAIL_T)
                else:
                    widths, rem = [], wcols - (TAIL_T if split_last else 0)
                    while rem > 0:
                        widths.append(min(MAIN_T, rem))
                        rem -= widths[-1]
                    if split_last:
                        widths.append(TAIL_T)
                g0 = 0
                for ti, gw in enumerate(widths):
                    if tail:
                        o_ps = tail_tile()
                    else:
                        o_ps = pout_pool.tile([P, gw], mybir.dt.float32, tag="o_ps")
                        o_ps = o_ps[:]
                    for j0 in range(0, gw, NMM):
                        jw = min(NMM, gw - j0)
                        nc.tensor.matmul(
                            o_ps[:, j0 : j0 + jw],
                            wT[:],
                            v_sb[:, g0 + j0 : g0 + j0 + jw],
                            start=True,
                            stop=True,
                        )
                    # ScalarE (after its tanh stream ends) takes the tail
                    # chunk plus the last 512 of the N_DEFER chunks before
                    # it; DVE takes the rest.
                    if tail or (split_last and ti == len(widths) - 1):
                        deferred.append((out_sb[:, g0 : g0 + gw], o_ps))
                    else:
                        nc.vector.tensor_copy(out_sb[:, g0 : g0 + gw], o_ps)
                    g0 += gw
                # Filler matmuls keep the PE stream dense (HAM k=8).
                for _ in range(FILLERS.get(ci, 0)):
                    t = tail_tile()
                    nc.tensor.matmul(
                        t[:, :P], warm[:], warm[:], start=True, stop=True
                    )
                if ci < len(CHUNKS) - N_TAIL - N_DEFER:
                    nc.gpsimd.dma_start(
                        out=outT[:, base : base + wcols], in_=out_sb[:]
                    )
                else:
                    # Chunks with a deferred eviction store only after the
                    # ScalarE drain completes their out tile.
                    stores.append((base, wcols, out_sb))

            # ScalarE drain (tanh done): evict tail slices, store on the
            # idle sync ring.
            for dst, o_ps in deferred:
                nc.scalar.copy(dst, o_ps)
            for base, wcols, out_sb in stores:
                nc.sync.dma_start(out=outT[:, base : base + wcols], in_=out_sb[:])

            # Hold the PE's k=8 activity state through the store drain and
            # into the framework's semaphore-reset storm (its Tensor-side
            # resets run ~2x faster at k=8).
            for wi in range(N_DUMMY):
                t = tail_tile()
                nc.tensor.matmul(t[:, :P], warm[:], warm[:], start=True, stop=True)
    nc.finalize()
    return nc


def kernel(x, coef, tanh_range):
    global LAST_RESULT
    x = np.asarray(x, dtype=np.float32)
    coef = np.asarray(coef, dtype=np.float32)
    t = float(np.asarray(tanh_range))
    assert x.shape == (B_FULL, I_DIM), x.shape
    assert coef.shape == (O_DIM, I_DIM, P_NUM), coef.shape

    # [identity | p-major coef blocks]: block p is the [i, o] slice.
    coefT = np.empty((I_DIM, COEF_COLS), dtype=np.float16)
    coefT[:, :P] = np.eye(P, dtype=np.float16)
    coefT[:, P:] = (
        coef.transpose(1, 2, 0).astype(np.float16).reshape(I_DIM, CW)
    )
    nc = build_bass(t)
    xt_full = np.ascontiguousarray(x.T).astype(ml_dtypes.float8_e4m3)
    in_maps = [
        {"xt": np.ascontiguousarray(xt_full[:, k * B_CORE : (k + 1) * B_CORE]),
         "coefT": coefT}
        for k in range(N_CORES)
    ]
    if os.environ.get("BASS_TRACE"):
        _ensure_ntff_hook()
    res = run_bass_kernel_spmd(nc, in_maps, core_ids=list(range(N_CORES)))
    LAST_RESULT = res
    return np.concatenate(
        [r["outT"].astype(np.float32).T for r in res.results], axis=0
    )


# revision 24
# speedup vs baseline: 1.1915x; 1.0606x over previous
"""Trainium2 Bass kernel for nn_LinearLayer_45243185496808.

Computes out[b,o] = sum_i tanh(x[b,i]*t) * (sum_p coef[o,i,p]) with
B=131072, I=O=128, P_NUM=16, data-parallel over batch on 8 NeuronCores.

Host-side staging (layout only, no module math):
  - xt: per-core x shard cast to f16 and laid out transposed+permuted
    [i=128, b] so that (a) loads are contiguous 2-4KB runs per partition,
    (b) the device needs zero PE transposes (the batch slice is loaded
    straight into the matmul stationary), and (c) each PSUM output slice
    lands directly in the contiguous-run store layout.
  - coefT: coef cast to f16 and laid out p-major [i, (p, o)] so each p
    is a contiguous [i, o] block.

Per-core device pipeline (B_CORE=16384 rows, ~41us HW incl the fixed
~9us framework semaphore-reset epilogue and ~3.5us launch):
  - prefetch: coef halves then all 12 x chunks issued up front on the
    HWDGE ring; every x tile stays resident (4 MiB = 32 KiB/partition)
    so loads never wait on compute.
  - prelude: wT[i,o] = sum_p coef as 16 identity matmuls accumulating
    in PSUM (I.T @ block_p = block_p) on the otherwise-idle PE, which
    doubles as HAM clock warmup; one small DVE cast -> wT f16.
  - per chunk: ScalarE tanh (SBUF->SBUF f16, the ~17us pole) -> per
    128-col slice one LDWEIGHTS(v slice f16, FWL)+MATMUL (rhs=wT,
    N=128, ~12us total) -> PSUM f32 -> eviction cast to f16 out tile
    (DVE 1x, the ~18us pole; the last two chunks evict on ScalarE)
    -> SWDGE store (separate ring from loads; tail stores on HWDGE).
  - output returns f16, host upcasts to f32.
HBM per core: 4 MiB x + 0.5 MiB coef read + 4 MiB out write ~ 24.4us
of DMA under the ScalarE/DVE dual-engine equilibrium.
"""

import os
import sys
import types

import ml_dtypes
import numpy as np

import concourse.bass as bass
import concourse.mybir as mybir
import concourse.tile as tile
from concourse import bacc, masks
from concourse.bass_utils import run_bass_kernel_spmd


def _ensure_ntff_hook():
    """Register the axon NTFF profile hook if the image lacks antenv.axon_hooks.

    Only needed for BASS_TRACE=1 profiling runs; harmless otherwise."""
    if "antenv.axon_hooks" in sys.modules:
        return
    try:
        from antenv.axon_hooks import get_axon_ntff_profile_hook  # noqa: F401

        return  # real module importable
    except ImportError:
        pass
    hook = None
    try:
        from trn_agent_boot.trn_boot import _ntff_profile_via_ctypes

        so_path = "/opt/axon/libaxon_pjrt.so"
        if os.path.exists(so_path):
            hook = _ntff_profile_via_ctypes(so_path)
    except Exception:
        hook = None
    mod = types.ModuleType("antenv.axon_hooks")
    mod.get_axon_ntff_profile_hook = lambda: hook
    mod.set_axon_ntff_profile_hook = lambda h: None
    sys.modules["antenv.axon_hooks"] = mod

N_CORES = 8
B_FULL = 131072
I_DIM = 128
O_DIM = 128
P_NUM = 16
P = 128                     # SBUF partitions
B_CORE = B_FULL // N_CORES  # 16384

# Column chunks of xt (base, width). Small leading chunks get the tanh
# stream started while the coef loads and w-reduction are still in
# flight; small trailing chunks shorten the drain tail. Each chunk is
# contiguous per partition.
CHUNKS = [(0, 256), (256, 768), (1024, 1024), (2048, 2048), (4096, 2048),
          (6144, 2048), (8192, 2048), (10240, 2048), (12288, 2048),
          (14336, 1024), (15360, 768), (16128, 256)]
assert CHUNKS[-1][0] + CHUNKS[-1][1] == B_CORE
assert all(a + w == b for (a, w), (b, _) in zip(CHUNKS, CHUNKS[1:]))
assert all(w % P == 0 for _, w in CHUNKS)

LAST_RESULT = None  # BassKernelResults of the most recent run (for test.py)


def build_bass(tanh_scale: float) -> bass.Bass:
    nc = bacc.Bacc("TRN2", target_bir_lowering=False)
    xt = nc.dram_tensor("xt", [P, B_CORE], mybir.dt.float8e4, kind="ExternalInput")
    coefT = nc.dram_tensor(
        "coefT", [I_DIM, O_DIM * P_NUM], mybir.dt.float16, kind="ExternalInput"
    )
    out = nc.dram_tensor("out", [B_CORE, O_DIM], mybir.dt.float16, kind="ExternalOutput")

    with tile.TileContext(nc) as tc:
        with (
            tc.tile_pool(name="consts", bufs=1) as consts,
            tc.tile_pool(name="xin", bufs=1) as xin_pool,
            tc.tile_pool(name="vals", bufs=6) as vals_pool,
            tc.tile_pool(name="outp", bufs=6) as out_pool,
            tc.tile_pool(name="pout", bufs=4, space="PSUM") as pout_pool,
        ):
            # --- prefetch: every load is issued before any compute ---
            # Tiny chunk 0 first (unblocks the first tanh), then coef in two
            # halves (each half's reduction tree starts as soon as it lands),
            # then the rest of x. All on the fast HWDGE ring; all x tiles
            # stay resident (4 MiB = 32 KiB/partition), so loads never wait
            # on compute and HBM stays busy end to end.
            identity_h = consts.tile([P, P], mybir.dt.float16)
            masks.make_identity(nc, identity_h[:])

            x_tiles = [None] * len(CHUNKS)

            def load_chunk(ci, eng):
                base, wcols = CHUNKS[ci]
                x_sb = xin_pool.tile([P, wcols], mybir.dt.float8e4, tag=f"x{ci}")
                eng.dma_start(out=x_sb[:], in_=xt[:, base : base + wcols])
                x_tiles[ci] = x_sb

            half = O_DIM * P_NUM // 2
            coef_halves = []

            def load_coef_half(hi, eng):
                csb = consts.tile([P, half], mybir.dt.float16, tag=f"coef{hi}")
                eng.dma_start(
                    out=csb[:], in_=coefT[:, hi * half : (hi + 1) * half]
                )
                coef_halves.append(csb)

            # All loads ride the HWDGE ring: the coef halves first (they
            # gate the w-accumulation matmuls and with them the whole
            # eviction stream), then every x chunk. Splitting loads onto
            # the SWDGE ring measured consistently slower.
            load_coef_half(0, nc.sync)
            load_coef_half(1, nc.sync)
            for ci in range(len(CHUNKS)):
                load_chunk(ci, nc.sync)

            # PE warmup on the identity while the DMAs are in flight, so
            # HAM reaches K=8/8 before the real work.
            for wi in range(2):
                wm_ps = pout_pool.tile([P, 8 * O_DIM], mybir.dt.float32, tag="o_ps")
                for wj in range(4):
                    nc.tensor.matmul(
                        wm_ps[:, wj * P : (wj + 1) * P],
                        identity_h[:],
                        identity_h[:],
                        start=True,
                        stop=True,
                    )

            # wT[i,o] = sum_p coefT as 16 identity matmuls accumulating in
            # PSUM: coefT is staged p-major so each p is a [i,o] block, and
            # I.T @ block = block. Runs on the otherwise-idle PE (doubling
            # as HAM warmup), identity stays loaded, first 8 accumulate
            # while the second coef half is still loading.
            w_big = pout_pool.tile([P, 8 * O_DIM], mybir.dt.float32, tag="o_ps")
            w_ps = w_big[:, :O_DIM]
            for k in range(P_NUM):
                csb = coef_halves[k // 8]
                kk = k % 8
                nc.tensor.matmul(
                    w_ps,
                    identity_h[:],
                    csb[:, kk * O_DIM : (kk + 1) * O_DIM],
                    start=(k == 0),
                    stop=(k == P_NUM - 1),
                )
            wT = consts.tile([P, O_DIM], mybir.dt.float16)
            nc.vector.tensor_copy(wT[:], w_ps)

            # --- main loop ---
            for ci, (base, wcols) in enumerate(CHUNKS):
                rpp = wcols // P  # output rows per partition for this chunk
                v_sb = vals_pool.tile([P, wcols], mybir.dt.float16, tag="v_sb")
                nc.scalar.activation(
                    v_sb[:],
                    x_tiles[ci][:],
                    mybir.ActivationFunctionType.Tanh,
                    scale=tanh_scale,
                )
                out_sb = out_pool.tile([P, wcols], mybir.dt.float16, tag="out_sb")
                out_view = out[base : base + wcols, :].rearrange(
                    "(p r) d -> p (r d)", p=P
                )
                # PSUM tiles span two banks (8 slices) to halve the
                # number of DVE eviction casts.
                for g0 in range(0, rpp, 8):
                    gw = min(8, rpp - g0)
                    o_ps = pout_pool.tile(
                        [P, gw * O_DIM], mybir.dt.float32, tag="o_ps"
                    )
                    for j in range(gw):
                        s = g0 + j
                        nc.tensor.matmul(
                            o_ps[:, j * O_DIM : (j + 1) * O_DIM],
                            v_sb[:, s * P : (s + 1) * P],
                            wT[:],
                            start=True,
                            stop=True,
                        )
                    # The tail chunks evict on ScalarE (its tanh stream is
                    # done by then, while DVE is still the eviction pacer).
                    if ci >= len(CHUNKS) - 2:
                        nc.scalar.copy(
                            out_sb[:, g0 * O_DIM : (g0 + gw) * O_DIM], o_ps[:]
                        )
                    else:
                        nc.vector.tensor_copy(
                            out_sb[:, g0 * O_DIM : (g0 + gw) * O_DIM], o_ps[:]
                        )
                # Stores ride the SWDGE ring so they never queue behind
                # loads; the tail stores use the by-then-idle HWDGE ring
                # (lower completion latency).
                (nc.sync if ci >= len(CHUNKS) - 2 else nc.gpsimd).dma_start(
                    out=out_view, in_=out_sb[:]
                )
    nc.finalize()
    return nc


def _stage_xt(x_core: np.ndarray) -> np.ndarray:
    """Pack a [B_CORE, I] f32 shard into the [I, B_CORE] fp8e4 device layout.

    Within each chunk of W columns (W/128 rows per partition), device
    column base + s*128 + p holds original row base + p*(W/128) + s, so
    each matmul output slice lands in the contiguous-run store layout.
    fp8 (TRN e4m3, max 240) halves the x HBM read vs f16; the tanh
    ACTIVATE reads fp8 at the same 1x rate and the end-to-end rel err is
    ~1.5e-2 vs the 2e-2 gate (fp8 quantization of x dominates).
    """
    xt = np.empty((I_DIM, B_CORE), dtype=ml_dtypes.float8_e4m3)
    for base, wcols in CHUNKS:
        rpp = wcols // P
        blk = x_core[base : base + wcols].reshape(P, rpp, I_DIM)  # [p, s, i]
        xt[:, base : base + wcols] = (
            blk.transpose(2, 1, 0).astype(ml_dtypes.float8_e4m3).reshape(I_DIM, wcols)
        )
    return xt


def kernel(x, coef, tanh_range):
    global LAST_RESULT
    x = np.asarray(x, dtype=np.float32)
    coef = np.asarray(coef, dtype=np.float32)
    t = float(np.asarray(tanh_range))
    assert x.shape == (B_FULL, I_DIM), x.shape
    assert coef.shape == (O_DIM, I_DIM, P_NUM), coef.shape

    coefT = np.ascontiguousarray(
        coef.transpose(1, 2, 0).astype(np.float16).reshape(I_DIM, P_NUM * O_DIM)
    )
    nc = build_bass(t)
    in_maps = [
        {"xt": _stage_xt(x[k * B_CORE : (k + 1) * B_CORE]), "coefT": coefT}
        for k in range(N_CORES)
    ]
    if os.environ.get("BASS_TRACE"):
        _ensure_ntff_hook()
    res = run_bass_kernel_spmd(nc, in_maps, core_ids=list(range(N_CORES)))
    LAST_RESULT = res
    return np.concatenate(
        [r["out"].astype(np.float32) for r in res.results], axis=0
    )



# revision 25
# speedup vs baseline: 1.2261x; 1.0290x over previous
"""Trainium2 Bass kernel for nn_LinearLayer_45243185496808.

Computes out[b,o] = sum_i tanh(x[b,i]*t) * (sum_p coef[o,i,p]) with
B=131072, I=O=128, P_NUM=16, data-parallel over batch on 8 NeuronCores.

Host-side staging (layout only, no module math):
  - xt: per-core x shard cast to f16 and laid out transposed+permuted
    [i=128, b] so that (a) loads are contiguous 2-4KB runs per partition,
    (b) the device needs zero PE transposes (the batch slice is loaded
    straight into the matmul stationary), and (c) each PSUM output slice
    lands directly in the contiguous-run store layout.
  - coefT: coef cast to f16 and laid out p-major [i, (p, o)] so each p
    is a contiguous [i, o] block.

Per-core device pipeline (B_CORE=16384 rows, ~41us HW incl the fixed
~9us framework semaphore-reset epilogue and ~3.5us launch):
  - prefetch: coef halves then all 12 x chunks issued up front on the
    HWDGE ring; every x tile stays resident (4 MiB = 32 KiB/partition)
    so loads never wait on compute.
  - prelude: wT[i,o] = sum_p coef as 16 identity matmuls accumulating
    in PSUM (I.T @ block_p = block_p) on the otherwise-idle PE, which
    doubles as HAM clock warmup; one small DVE cast -> wT f16.
  - per chunk: ScalarE tanh (SBUF->SBUF f16, the ~17us pole) -> per
    128-col slice one LDWEIGHTS(v slice f16, FWL)+MATMUL (rhs=wT,
    N=128, ~12us total) -> PSUM f32 -> eviction cast to f16 out tile
    (DVE 1x, the ~18us pole; the last two chunks evict on ScalarE)
    -> SWDGE store (separate ring from loads; tail stores on HWDGE).
  - output returns f16, host upcasts to f32.
HBM per core: 4 MiB x + 0.5 MiB coef read + 4 MiB out write ~ 24.4us
of DMA under the ScalarE/DVE dual-engine equilibrium.
"""

import os
import sys
import types

import numpy as np

import concourse.bass as bass
import concourse.mybir as mybir
import concourse.tile as tile
from concourse import bacc, masks
from concourse.bass_utils import run_bass_kernel_spmd


def _ensure_ntff_hook():
    """Register the axon NTFF profile hook if the image lacks antenv.axon_hooks.

    Only needed for BASS_TRACE=1 profiling runs; harmless otherwise."""
    if "antenv.axon_hooks" in sys.modules:
        return
    try:
        from antenv.axon_hooks import get_axon_ntff_profile_hook  # noqa: F401

        return  # real module importable
    except ImportError:
        pass
    hook = None
    try:
        from trn_agent_boot.trn_boot import _ntff_profile_via_ctypes

        so_path = "/opt/axon/libaxon_pjrt.so"
        if os.path.exists(so_path):
            hook = _ntff_profile_via_ctypes(so_path)
    except Exception:
        hook = None
    mod = types.ModuleType("antenv.axon_hooks")
    mod.get_axon_ntff_profile_hook = lambda: hook
    mod.set_axon_ntff_profile_hook = lambda h: None
    sys.modules["antenv.axon_hooks"] = mod

N_CORES = 8
B_FULL = 131072
I_DIM = 128
O_DIM = 128
P_NUM = 16
P = 128                     # SBUF partitions
B_CORE = B_FULL // N_CORES  # 16384

# Column chunks of xt (base, width). Small leading chunks get the tanh
# stream started while the coef loads and w-reduction are still in
# flight; small trailing chunks shorten the drain tail. Each chunk is
# contiguous per partition.
CHUNKS = [(0, 256), (256, 768), (1024, 1024), (2048, 2048), (4096, 2048),
          (6144, 2048), (8192, 2048), (10240, 2048), (12288, 2048),
          (14336, 1024), (15360, 768), (16128, 256)]
assert CHUNKS[-1][0] + CHUNKS[-1][1] == B_CORE
assert all(a + w == b for (a, w), (b, _) in zip(CHUNKS, CHUNKS[1:]))
assert all(w % P == 0 for _, w in CHUNKS)

LAST_RESULT = None  # BassKernelResults of the most recent run (for test.py)


def build_bass(tanh_scale: float) -> bass.Bass:
    nc = bacc.Bacc("TRN2", target_bir_lowering=False)
    xt = nc.dram_tensor("xt", [P, B_CORE], mybir.dt.float16, kind="ExternalInput")
    coefT = nc.dram_tensor(
        "coefT", [I_DIM, O_DIM * P_NUM], mybir.dt.float16, kind="ExternalInput"
    )
    out = nc.dram_tensor("out", [B_CORE, O_DIM], mybir.dt.float16, kind="ExternalOutput")

    with tile.TileContext(nc) as tc:
        with (
            tc.tile_pool(name="consts", bufs=1) as consts,
            tc.tile_pool(name="xin", bufs=1) as xin_pool,
            tc.tile_pool(name="vals", bufs=6) as vals_pool,
            tc.tile_pool(name="outp", bufs=6) as out_pool,
            tc.tile_pool(name="pout", bufs=4, space="PSUM") as pout_pool,
        ):
            # --- prefetch: every load is issued before any compute ---
            # Tiny chunk 0 first (unblocks the first tanh), then coef in two
            # halves (each half's reduction tree starts as soon as it lands),
            # then the rest of x. All on the fast HWDGE ring; all x tiles
            # stay resident (4 MiB = 32 KiB/partition), so loads never wait
            # on compute and HBM stays busy end to end.
            identity_h = consts.tile([P, P], mybir.dt.float16)
            masks.make_identity(nc, identity_h[:])

            x_tiles = [None] * len(CHUNKS)

            def load_chunk(ci, eng):
                base, wcols = CHUNKS[ci]
                x_sb = xin_pool.tile([P, wcols], mybir.dt.float16, tag=f"x{ci}")
                eng.dma_start(out=x_sb[:], in_=xt[:, base : base + wcols])
                x_tiles[ci] = x_sb

            half = O_DIM * P_NUM // 2
            coef_halves = []

            def load_coef_half(hi, eng):
                csb = consts.tile([P, half], mybir.dt.float16, tag=f"coef{hi}")
                eng.dma_start(
                    out=csb[:], in_=coefT[:, hi * half : (hi + 1) * half]
                )
                coef_halves.append(csb)

            # All loads ride the HWDGE ring: the coef halves first (they
            # gate the w-accumulation matmuls and with them the whole
            # eviction stream), then every x chunk. Splitting loads onto
            # the SWDGE ring measured consistently slower.
            load_coef_half(0, nc.sync)
            load_coef_half(1, nc.sync)
            for ci in range(len(CHUNKS)):
                load_chunk(ci, nc.sync)

            # PE warmup on the identity while the DMAs are in flight, so
            # HAM reaches K=8/8 before the real work.
            for wi in range(2):
                wm_ps = pout_pool.tile([P, 8 * O_DIM], mybir.dt.float32, tag="o_ps")
                for wj in range(4):
                    nc.tensor.matmul(
                        wm_ps[:, wj * P : (wj + 1) * P],
                        identity_h[:],
                        identity_h[:],
                        start=True,
                        stop=True,
                    )

            # wT[i,o] = sum_p coefT as 16 identity matmuls accumulating in
            # PSUM: coefT is staged p-major so each p is a [i,o] block, and
            # I.T @ block = block. Runs on the otherwise-idle PE (doubling
            # as HAM warmup), identity stays loaded, first 8 accumulate
            # while the second coef half is still loading.
            w_big = pout_pool.tile([P, 8 * O_DIM], mybir.dt.float32, tag="o_ps")
            w_ps = w_big[:, :O_DIM]
            for k in range(P_NUM):
                csb = coef_halves[k // 8]
                kk = k % 8
                nc.tensor.matmul(
                    w_ps,
                    identity_h[:],
                    csb[:, kk * O_DIM : (kk + 1) * O_DIM],
                    start=(k == 0),
                    stop=(k == P_NUM - 1),
                )
            wT = consts.tile([P, O_DIM], mybir.dt.float16)
            nc.vector.tensor_copy(wT[:], w_ps)

            # --- main loop ---
            for ci, (base, wcols) in enumerate(CHUNKS):
                rpp = wcols // P  # output rows per partition for this chunk
                v_sb = vals_pool.tile([P, wcols], mybir.dt.float16, tag="v_sb")
                nc.scalar.activation(
                    v_sb[:],
                    x_tiles[ci][:],
                    mybir.ActivationFunctionType.Tanh,
                    scale=tanh_scale,
                )
                out_sb = out_pool.tile([P, wcols], mybir.dt.float16, tag="out_sb")
                out_view = out[base : base + wcols, :].rearrange(
                    "(p r) d -> p (r d)", p=P
                )
                # PSUM tiles span two banks (8 slices) to halve the
                # number of DVE eviction casts.
                for g0 in range(0, rpp, 8):
                    gw = min(8, rpp - g0)
                    o_ps = pout_pool.tile(
                        [P, gw * O_DIM], mybir.dt.float32, tag="o_ps"
                    )
                    for j in range(gw):
                        s = g0 + j
                        nc.tensor.matmul(
                            o_ps[:, j * O_DIM : (j + 1) * O_DIM],
                            v_sb[:, s * P : (s + 1) * P],
                            wT[:],
                            start=True,
                            stop=True,
                        )
                    # The tail chunks evict on ScalarE (its tanh stream is
                    # done by then, while DVE is still the eviction pacer).
                    if ci >= len(CHUNKS) - 2:
                        nc.scalar.copy(
                            out_sb[:, g0 * O_DIM : (g0 + gw) * O_DIM], o_ps[:]
                        )
                    else:
                        nc.vector.tensor_copy(
                            out_sb[:, g0 * O_DIM : (g0 + gw) * O_DIM], o_ps[:]
                        )
                # Stores ride the SWDGE ring so they never queue behind
                # loads; the tail stores use the by-then-idle HWDGE ring
                # (lower completion latency).
                (nc.sync if ci >= len(CHUNKS) - 2 else nc.gpsimd).dma_start(
                    out=out_view, in_=out_sb[:]
                )
    nc.finalize()
    return nc


def _stage_xt(x_core: np.ndarray) -> np.ndarray:
    """Pack a [B_CORE, I] f32 shard into the [I, B_CORE] f16 device layout.

    Within each chunk of W columns (W/128 rows per partition), device
    column base + s*128 + p holds original row base + p*(W/128) + s, so
    each matmul output slice lands in the contiguous-run store layout.
    """
    xt = np.empty((I_DIM, B_CORE), dtype=np.float16)
    for base, wcols in CHUNKS:
        rpp = wcols // P
        blk = x_core[base : base + wcols].reshape(P, rpp, I_DIM)  # [p, s, i]
        xt[:, base : base + wcols] = (
            blk.transpose(2, 1, 0).astype(np.float16).reshape(I_DIM, wcols)
        )
    return xt


def kernel(x, coef, tanh_range):
    global LAST_RESULT
    x = np.asarray(x, dtype=np.float32)
    coef = np.asarray(coef, dtype=np.float32)
    t = float(np.asarray(tanh_range))
    assert x.shape == (B_FULL, I_DIM), x.shape
    assert coef.shape == (O_DIM, I_DIM, P_NUM), coef.shape

    coefT = np.ascontiguousarray(
        coef.transpose(1, 2, 0).astype(np.float16).reshape(I_DIM, P_NUM * O_DIM)
    )
    nc = build_bass(t)
    in_maps = [
        {"xt": _stage_xt(x[k * B_CORE : (k + 1) * B_CORE]), "coefT": coefT}
        for k in range(N_CORES)
    ]
    if os.environ.get("BASS_TRACE"):
        _ensure_ntff_hook()
    res = run_bass_kernel_spmd(nc, in_maps, core_ids=list(range(N_CORES)))
    LAST_RESULT = res
    return np.concatenate(
        [r["out"].astype(np.float32) for r in res.results], axis=0
    )

